# revision 51
# baseline (speedup 1.0000x reference)
"""Self-contained TRN2 Bass kernel for nn_Encoder_49065706389648.

Transformer encoder layer (B=8, S=2048, D=768, HID=1536), data-parallel:
one batch element per NeuronCore across 8 cores, weights replicated.

Attention block runs feature-major with fp8 DoubleRow matmuls.  The
post-attention segment (proj output, LayerNorms, MLP2 output) runs
TOKEN-major: the proj/MLP2 matmuls are emitted transposed so their
outputs land [token, feature] in fused [128,768] PSUM tiles (1.5 banks).
Residual adds run on the DVE (PSUM + residual -> z fp16 in SBUF, one op
per tile) instead of PE identity-matmuls.  LayerNorm stats live on the
partition dim (bn_stats/bn_aggr, scalar sqrt, DVE recip + apply).
V/proj biases fold into the host-side residual (bp' = bp + Wp@bv); the
MLP2 bias is added with a ones-row matmul.  Output is fp16 (cast to f32
on host).

Schedule notes (see _pipeline): DMA issues are dual-streamed (sync +
scalar HWDGE) with wq/xt quarters first; LN chains are emitted between
denom and attnv so scalar sqrts queue after the exps and applies finish
before transp needs them; transp separates the two ps3 scopes
(projT/mlp2T); LN1 of the last chunk is emitted an iteration early; the
last chunk's mlp2 interleaves per-group LN2+DMA to shorten the tail.

kernel(**inputs) takes the FULL unsharded inputs (as from setup_inputs())
and returns the FULL [8, 2048, 768] float32 output.
"""
import sys
sys.path.insert(0, '/opt/trn_rl_repo')

# ---------------------------------------------------------------- birpatch --
# This walrus build rejects instructions carrying more than ~1-2 semaphore
# waits ("Too many sync wait commands"). Split excess waits onto injected
# wait-only EventSemaphore instructions at the BIR JSON level.
import base64
import orjson
import zstandard

MAXW = 1

_counter = [0]


def split_waits(bir: dict, maxw: int = MAXW) -> int:
    nsplit = 0
    for fn in bir.get("functions", []):
        for blk in fn.get("blocks", []):
            insts = blk.get("instructions", [])
            new_insts = []
            for ins in insts:
                si = ins.get("sync_info")
                waits = (si or {}).get("on_wait") or []
                cap = {"Drain": 0}.get(ins.get("opcode"), maxw)
                if len(waits) > cap:
                    excess, keep = (waits, []) if cap == 0 else (waits[:-cap], waits[-cap:])
                    for i in range(0, len(excess), maxw):
                        _counter[0] += 1
                        new_insts.append({
                            "engine": ins["engine"],
                            "ins": [],
                            "outs": [],
                            "name": f"wsplit-{_counter[0]}",
                            "opcode": "EventSemaphore",
                            "sync_info": {
                                "on_update": [],
                                "on_wait": excess[i:i + maxw],
                            },
                            "debug": ins.get("debug", 0),
                        })
                    si["on_wait"] = keep
                    nsplit += 1
                new_insts.append(ins)
            blk["instructions"] = new_insts
    return nsplit


def install():
    import concourse.bass2jax as bass2jax
    import concourse.bass_utils as bass_utils
    if getattr(bass2jax, "_ant_wait_split_installed", False):
        return
    import os
    if os.environ.get("ANT_LDW_OPT", "0") == "1":
        _orig_run = bass_utils.run_command
        def _run(argv, **kw):
            argv = ["--enable-ldw-opt=true" if a == "--enable-ldw-opt=false"
                    else a for a in argv]
            return _orig_run(argv, **kw)
        bass_utils.run_command = _run

    def _patched(ant_bir_value: str) -> bytes:
        raw = zstandard.ZstdDecompressor().decompress(
            base64.standard_b64decode(ant_bir_value)
        )
        bir = orjson.loads(raw)
        n = split_waits(bir)
        if n:
            print(f"[birpatch] split waits on {n} instructions")
        return orjson.dumps(bir)

    bass2jax._decompress_ant_bir = _patched
    bass2jax._ant_wait_split_installed = True


# ----------------------------------------------------------------- builder --
import numpy as np
import ml_dtypes
import concourse.bass as bass
import concourse.mybir as mybir
import concourse.tile as tile

F32 = mybir.dt.float32
F16 = mybir.dt.float16
BF16 = mybir.dt.bfloat16
FP8 = mybir.dt.float8e4
PM = mybir.MatmulPerfMode
AF = mybir.ActivationFunctionType
OP = mybir.AluOpType

S, D, HID = 2048, 768, 1536
KD = D // 128      # 6   d-tiles
KH = HID // 128    # 12  hid-tiles
NK = S // 128      # 16  seq-tiles
CW = 512           # q-chunk width processed per pipeline pass
NQ = S // CW       # 4   chunks
NT = CW // 128     # 4   token blocks per chunk
EPS = 1e-12
ISCALE = float(1.0 / np.sqrt(D))


class Ctx:
    pass


def build(nc: bass.Bass, reps: int = 1, loop_n: int = 0):
    c = Ctx()
    c.nc = nc
    # ---- DRAM I/O ----
    c.xt_d = nc.dram_tensor("xt", [KD // 2, 128, 2, S], FP8,
                            kind="ExternalInput")
    # token-major residual: x[b] + bp' reshaped [S/128, 128, D]
    c.xres_d = nc.dram_tensor("xres", [NK, 128, D], BF16,
                              kind="ExternalInput")
    c.wq_d = nc.dram_tensor("wq", [KD // 2, 128, 2, D], FP8,
                            kind="ExternalInput")
    c.wk_d = nc.dram_tensor("wk", [KD // 2, 128, 2, D], FP8,
                            kind="ExternalInput")
    c.wv_d = nc.dram_tensor("wv", [KD // 2, 128, 2, D], FP8,
                            kind="ExternalInput")
    c.wp_d = nc.dram_tensor("wp", [KD // 2, 128, 2, D], FP8,
                            kind="ExternalInput")
    c.w1_d = nc.dram_tensor("w1", [KD // 2, 128, 2, HID], FP8,
                            kind="ExternalInput")
    c.w2_d = nc.dram_tensor("w2", [KH // 2, 128, 2, D], FP8,
                            kind="ExternalInput")
    c.bq_d = nc.dram_tensor("bq", [128, KD], F32, kind="ExternalInput")
    c.bk_d = nc.dram_tensor("bk", [128, KD], F32, kind="ExternalInput")
    c.b1_d = nc.dram_tensor("b1", [128, KH], F32, kind="ExternalInput")
    c.b2row_d = nc.dram_tensor("b2row", [1, D], BF16, kind="ExternalInput")
    c.ident_d = nc.dram_tensor("ident", [128, 128], BF16,
                               kind="ExternalInput")
    # token-major output: [S/128, 128, D] (fp16; host casts to f32)
    c.yt_d = nc.dram_tensor("yt", [NK, 128, D], F16, kind="ExternalOutput")

    with tile.TileContext(nc) as tc:
        with tc.tile_pool(name="sb", bufs=1) as sb, \
             tc.tile_pool(name="ps", bufs=1, space=bass.MemorySpace.PSUM) as ps:
            c.tc, c.sb, c.ps = tc, sb, ps

            c.ones8 = sb.tile([128, 2, 128], FP8, tag="ones8")
            nc.vector.memset(c.ones8[:], 1.0)
            c.ones1p = sb.tile([1, 128], BF16, tag="ones1p")
            nc.vector.memset(c.ones1p[:], 1.0)
            c.eps_t = sb.tile([128, 1], F32, tag="eps")
            nc.vector.memset(c.eps_t[:], EPS)

            # bias/const tiles; their DMAs are issued inside _phase_a on the
            # scalar HWDGE stream, after the critical wq/xt loads.
            c.bq_t = sb.tile([128, KD], F32, tag="bias", bufs=3)
            c.bk_t = sb.tile([128, KD], F32, tag="bias", bufs=3)
            c.b1_t = sb.tile([128, KH], F32, tag="bias", bufs=3)
            c.b2row_t = sb.tile([1, D], BF16, tag="b2row")
            c.ident_t = sb.tile([128, 128], BF16, tag="ident")

            if loop_n:
                with tc.For_i(0, loop_n, 1) as _i:
                    _pipeline(c)
            else:
                for _ in range(reps):
                    _pipeline(c)
    return nc


def _pipeline(c):
    """Per-iteration emission order is tuned so that: LN stats+sqrt get
    early scalar-queue slots (inputs ready at iteration start), the exps
    of scores(q+1) follow immediately, transp(q) separates the two ps3
    scopes (projT/mlp2T) on the PE, and projT(q+1)'s z1 adds free ps3
    banks during mlp1(q)."""
    _phase_a(c)
    st = [Ctx() for _ in range(NQ)]   # per-chunk state
    _scores(c, st[0], 0)
    _denom(c, st[0], 0)
    _attnv(c, st[0], 0)
    _projT(c, st[0], 0)
    for q in range(NQ):
        if q + 1 < NQ:
            _scores(c, st[q + 1], q + 1)
            _denom(c, st[q + 1], q + 1)
        # LN chains after the exps (scalar) but before the attnT mults
        # (vector): sqrts retire as exps drain; applies land mid-scores,
        # long before transp(q) needs hbT(q).
        if not getattr(st[q], "ln1_done", False):
            _ln1(c, st[q], q)
        if q >= 1 and q + 1 < NQ:
            _ln2_stats(c, st[q - 1], q - 1)
            _ln2_apply(c, st[q - 1], q - 1)
        if q + 1 < NQ:
            _attnv(c, st[q + 1], q + 1)
        _transp(c, st[q], q)
        if q + 1 < NQ:
            _projT(c, st[q + 1], q + 1)
            if q + 2 == NQ:
                # last chunk's LN1 emitted an iteration early so
                # transp(last) never waits on it
                _ln1(c, st[q + 1], q + 1)
                st[q + 1].ln1_done = True
        _mlp1(c, st[q], q)
        if q == NQ - 1:
            # no exps in the last iteration: LN2(q-1) emitted after the
            # gelus so neither transp copies nor gelus queue behind it
            _ln2_stats(c, st[q - 1], q - 1)
            _ln2_apply(c, st[q - 1], q - 1)
            _mlp2T_last(c, st[q], q)
        else:
            _mlp2T(c, st[q], q)


def _phase_a(c):
    nc, sb, ps = c.nc, c.sb, c.ps
    c.xt = [sb.tile([128, 2, S], FP8, tag="xt", bufs=KD // 2, name=f"xt{i}")
            for i in range(KD // 2)]
    wq = [sb.tile([128, 2, D], FP8, tag="w8", bufs=9, name=f"wq{i}")
          for i in range(KD // 2)]
    wk = [sb.tile([128, 2, D], FP8, tag="w8", bufs=9, name=f"wk{i}")
          for i in range(KD // 2)]
    wv = [sb.tile([128, 2, D], FP8, tag="w8", bufs=9, name=f"wv{i}")
          for i in range(KD // 2)]
    c.wp = [sb.tile([128, 2, D], FP8, tag="wp", bufs=KD // 2, name=f"wp{i}")
            for i in range(KD // 2)]
    c.w1 = [sb.tile([128, 2, HID], FP8, tag="w1536", bufs=KD // 2,
                    name=f"w1{i}") for i in range(KD // 2)]
    c.w2 = [sb.tile([128, 2, D], FP8, tag="w768", bufs=KH // 2,
                    name=f"w2{i}") for i in range(KH // 2)]

    # --- sync HWDGE stream: critical-path loads first (wq + xt first
    # halves interleaved; h0 as quarters for queue parallelism), then the
    # weights needed later (wv, w1, w2), spread over 8 HW queues.
    Q4 = S // 4
    for i in range(KD // 2):
        nc.sync.dma_start(wq[i][:], c.wq_d[i])
        nc.sync.dma_start(c.xt[i][:, :, 0:Q4], c.xt_d[i][:, :, 0:Q4])
        nc.sync.dma_start(c.xt[i][:, :, Q4:2 * Q4], c.xt_d[i][:, :, Q4:2 * Q4])
    for i in range(KD // 2):
        nc.sync.dma_start(c.xt[i][:, :, 2 * Q4:3 * Q4],
                          c.xt_d[i][:, :, 2 * Q4:3 * Q4])
        nc.sync.dma_start(c.xt[i][:, :, 3 * Q4:S], c.xt_d[i][:, :, 3 * Q4:S])

    # --- scalar HWDGE stream (parallel): wk + biases + remaining weights.
    for i in range(KD // 2):
        nc.scalar.dma_start(wk[i][:], c.wk_d[i])
    nc.scalar.dma_start(c.bq_t[:], c.bq_d[:])
    nc.scalar.dma_start(c.bk_t[:], c.bk_d[:])
    nc.scalar.dma_start(c.b1_t[:], c.b1_d[:])
    nc.scalar.dma_start(c.b2row_t[:], c.b2row_d[:])
    nc.scalar.dma_start(c.ident_t[:], c.ident_d[:])
    for i in range(KD // 2):
        nc.scalar.dma_start(c.wp[i][:], c.wp_d[i])

    c.QT = [sb.tile([128, 2, S], FP8, tag="qkt", bufs=6, name=f"QT{i}")
            for i in range(KD // 2)]
    c.KT = [sb.tile([128, 2, S], FP8, tag="qkt", bufs=6, name=f"KT{i}")
            for i in range(KD // 2)]
    c.V = [sb.tile([128, 2, D], FP8, tag="v768", bufs=NK // 2, name=f"V{i}")
           for i in range(NK // 2)]

    with nc.named_scope("qk_proj"):
        for wi, (W, BIAS, OUT) in enumerate(((wq, c.bq_t, c.QT),
                                             (wk, c.bk_t, c.KT))):
            if wi == 1:
                # wv/w1/w2 issues deferred past the head's critical DMA
                # window (xt/wq/wk); they are not needed until v_proj+.
                for i in range(KD // 2):
                    nc.sync.dma_start(wv[i][:], c.wv_d[i])
                for i in range(KD // 2):
                    nc.sync.dma_start(c.w1[i][:], c.w1_d[i])
                for i in range(KH // 2):
                    nc.sync.dma_start(c.w2[i][:], c.w2_d[i])
            for qh in range(2):
                for e in range(KD):
                    pq = [ps.tile([128, CW], F32, tag="ps1", bufs=4,
                                  name=f"pq{e}_{qh}_{qc}") for qc in range(2)]
                    for k in range(KD // 2):
                        for qc in range(2):
                            nc.tensor.matmul(
                                pq[qc][:],
                                W[k][:, :, e * 128:(e + 1) * 128],
                                c.xt[k][:, :, (2 * qh + qc) * CW:
                                        (2 * qh + qc + 1) * CW],
                                start=(k == 0), stop=(k == KD // 2 - 1),
                                perf_mode=PM.DoubleRow)
                    for qc in range(2):
                        nc.scalar.activation(
                            OUT[e // 2][:, e % 2,
                                        (2 * qh + qc) * CW:
                                        (2 * qh + qc + 1) * CW],
                            pq[qc][:], AF.Identity, bias=BIAS[:, e:e + 1])

    with nc.named_scope("v_proj"):
        for s in range(NK):
            pv = ps.tile([128, D], F32, tag="ps3", bufs=2, name=f"pv{s}")
            for k in range(KD // 2):
                nc.tensor.matmul(pv[:, 0:512],
                                 c.xt[k][:, :, s * 128:(s + 1) * 128],
                                 wv[k][:, :, 0:512],
                                 start=(k == 0), stop=(k == KD // 2 - 1),
                                 perf_mode=PM.DoubleRow)
                nc.tensor.matmul(pv[:, 512:768],
                                 c.xt[k][:, :, s * 128:(s + 1) * 128],
                                 wv[k][:, :, 512:768],
                                 start=(k == 0), stop=(k == KD // 2 - 1),
                                 perf_mode=PM.DoubleRow)
            nc.vector.tensor_copy(c.V[s // 2][:, s % 2, :], pv[:])


def _scores(c, s, q):
    nc, sb, ps = c.nc, c.sb, c.ps
    cs = slice(q * CW, (q + 1) * CW)
    s.PT = [sb.tile([128, 2, CW], FP8, tag="pt", bufs=10,
                    name=f"PT{q}_{k}") for k in range(NK // 2)]
    with nc.named_scope(f"scores{q}"):
        for k in range(NK):
            pss = ps.tile([128, CW], F32, tag="ps1", bufs=4,
                          name=f"pss{q}_{k}")
            for i in range(KD // 2):
                nc.tensor.matmul(pss[:],
                                 c.KT[i][:, :, k * 128:(k + 1) * 128],
                                 c.QT[i][:, :, cs],
                                 start=(i == 0), stop=(i == KD // 2 - 1),
                                 perf_mode=PM.DoubleRow)
            pt_half = s.PT[k // 2][:, k % 2, :]
            nc.scalar.activation(pt_half, pss[:], AF.Exp, scale=ISCALE)


def _denom(c, s, q):
    nc, sb, ps = c.nc, c.sb, c.ps
    s.rden = sb.tile([128, CW], F32, tag="f32c", bufs=3, name=f"rden{q}")
    with nc.named_scope(f"denom{q}"):
        psd = ps.tile([128, CW], F32, tag="ps1", bufs=4, name=f"psd{q}")
        for j in range(NK // 2):
            nc.tensor.matmul(psd[:], c.ones8[:], s.PT[j][:, :, :],
                             start=(j == 0), stop=(j == NK // 2 - 1),
                             perf_mode=PM.DoubleRow)
        nc.vector.reciprocal(s.rden[:], psd[:])


def _attnv(c, s, q):
    nc, sb, ps = c.nc, c.sb, c.ps
    s.attnT = [sb.tile([128, 2, CW], FP8, tag="attc", bufs=6,
                       name=f"at{q}_{d}") for d in range(KD // 2)]
    # prefetch token-major residual tiles for this chunk
    s.xres = [sb.tile([128, D], BF16, tag="xres", bufs=2 * NT,
                      name=f"xr{q}_{t}") for t in range(NT)]
    for t in range(NT):
        nc.sync.dma_start(s.xres[t][:], c.xres_d[q * NT + t])
    with nc.named_scope(f"attnv{q}"):
        for d in range(KD):
            pa = ps.tile([128, CW], F32, tag="ps1", bufs=4, name=f"pa{q}_{d}")
            for j in range(NK // 2):
                nc.tensor.matmul(pa[:],
                                 c.V[j][:, :, d * 128:(d + 1) * 128],
                                 s.PT[j][:, :, :],
                                 start=(j == 0), stop=(j == NK // 2 - 1),
                                 perf_mode=PM.DoubleRow)
            nc.vector.tensor_tensor(s.attnT[d // 2][:, d % 2, :],
                                    pa[:], s.rden[:], op=OP.mult)


def _projT(c, s, q):
    """attn output projection, emitted transposed: out [token, feature].
    The residual (x + bp') is added on the DVE (PSUM + xres -> z fp16 in
    SBUF), freeing the PE of identity matmuls.  LN1 stats+apply run on the
    z tiles pairwise."""
    nc, sb, ps = c.nc, c.sb, c.ps
    s.hbT = [sb.tile([128, D], BF16, tag="hbt", bufs=8, name=f"hbT{q}_{t}")
             for t in range(NT)]
    s.z1 = [sb.tile([128, D], F16, tag="z1", bufs=8, name=f"z1_{q}_{t}")
            for t in range(NT)]
    for pair in range(NT // 2):
        with nc.named_scope(f"projT{q}_{pair}"):
            for t in (2 * pair, 2 * pair + 1):
                pw = ps.tile([128, D], F32, tag="ps3", bufs=2,
                             name=f"pw{q}_{t}")
                for d in range(KD // 2):
                    nc.tensor.matmul(pw[:, 0:512],
                                     s.attnT[d][:, :, t * 128:(t + 1) * 128],
                                     c.wp[d][:, :, 0:512],
                                     start=(d == 0), stop=(d == KD // 2 - 1),
                                     perf_mode=PM.DoubleRow)
                    nc.tensor.matmul(pw[:, 512:768],
                                     s.attnT[d][:, :, t * 128:(t + 1) * 128],
                                     c.wp[d][:, :, 512:768],
                                     start=(d == 0), stop=(d == KD // 2 - 1),
                                     perf_mode=PM.DoubleRow)
                nc.vector.tensor_tensor(s.z1[t][:], pw[:], s.xres[t][:],
                                        op=OP.add)


def _ln_stats(c, q, which, zall, groups):
    """Stats half of token-major LN: bn_stats/bn_aggr + sqrt + recip per
    group of 1-2 tiles.  Returns [(ts, MV, R)] for _ln_apply."""
    nc, sb = c.nc, c.sb
    mrs = []
    for ts in groups:
        g = ts[0]
        n = len(ts)
        MV = sb.tile([128, 2, 2], F32, tag="mv", bufs=24,
                     name=f"MV{q}_{which}_{g}")
        SD = sb.tile([128, 2], F32, tag="mv", bufs=24,
                     name=f"SD{q}_{which}_{g}")
        R = sb.tile([128, 2], F32, tag="mv", bufs=24,
                    name=f"R{q}_{which}_{g}")
        with nc.named_scope(f"ln{which}s_{q}_{g}"):
            for i, t in enumerate(ts):
                z = zall[t]
                st = sb.tile([128, 2, 6], F32, tag="st", bufs=8,
                             name=f"st{q}_{which}_{g}_{i}")
                nc.vector.bn_stats(st[:, 0, :], z[:, 0:512])
                nc.vector.bn_stats(st[:, 1, :], z[:, 512:768])
                nc.vector.bn_aggr(MV[:, i, :], st[:])
            nc.scalar.activation(SD[:, 0:n], MV[:, 0:n, 1], AF.Sqrt,
                                 bias=c.eps_t[:])
            nc.vector.reciprocal(R[:, 0:n], SD[:, 0:n])
        mrs.append((ts, MV, R))
    return mrs


def _ln_apply(c, q, which, zall, mrs, out_ap, post=None):
    nc = c.nc
    with nc.named_scope(f"ln{which}a_{q}"):
        for (ts, MV, R) in mrs:
            for i, t in enumerate(ts):
                nc.vector.tensor_scalar(out_ap(t), zall[t][:],
                                        MV[:, i, 0:1], R[:, i:i + 1],
                                        op0=OP.subtract, op1=OP.mult)
                if post is not None:
                    post(t)


def _ln1(c, s, q):
    mrs = _ln_stats(c, q, 1, s.z1, ((0, 1), (2, 3)))
    _ln_apply(c, q, 1, s.z1, mrs, lambda t: s.hbT[t][:])


def _transp(c, s, q):
    """Transpose hbT [token, feat] -> hb [feat, token] for the MLP1 matmul."""
    nc, sb, ps = c.nc, c.sb, c.ps
    s.hb = [sb.tile([128, 2, CW], FP8, tag="hb", bufs=4, name=f"hb{q}_{i}")
            for i in range(KD // 2)]
    with nc.named_scope(f"transp{q}"):
        for f in range(KD):
            tp = ps.tile([128, CW], BF16, tag="ps1", bufs=4,
                         name=f"tp{q}_{f}")
            for t in range(NT):
                nc.tensor.transpose(tp[:, t * 128:(t + 1) * 128],
                                    s.hbT[t][:, f * 128:(f + 1) * 128],
                                    c.ident_t[:])
            nc.vector.tensor_copy(s.hb[f // 2][:, f % 2, :], tp[:])


def _mlp1(c, s, q):
    nc, sb, ps = c.nc, c.sb, c.ps
    s.mlpb = [sb.tile([128, 2, CW], FP8, tag="mlpb", bufs=7,
                      name=f"mb{q}_{j}") for j in range(KH // 2)]
    with nc.named_scope(f"mlp1_{q}"):
        for h in range(KH):
            pm = ps.tile([128, CW], F32, tag="ps1", bufs=4, name=f"pm{q}_{h}")
            for k in range(KD // 2):
                nc.tensor.matmul(pm[:],
                                 c.w1[k][:, :, h * 128:(h + 1) * 128],
                                 s.hb[k][:],
                                 start=(k == 0), stop=(k == KD // 2 - 1),
                                 perf_mode=PM.DoubleRow)
            nc.scalar.activation(s.mlpb[h // 2][:, h % 2, :], pm[:], AF.Gelu,
                                 bias=c.b1_t[:, h:h + 1])


def _mlp2T(c, s, q):
    """MLP2, emitted transposed: out [token, feature]; +b2 via ones-row
    matmul in PSUM; the h residual is added on the DVE (PSUM + hbT -> z2
    fp16).  LN2 runs later (see _ln2) so its scalar sqrt does not block
    the next chunk's exps."""
    nc, sb, ps = c.nc, c.sb, c.ps
    s.z2 = [sb.tile([128, D], F16, tag="z2", bufs=8, name=f"z2_{q}_{t}")
            for t in range(NT)]
    with nc.named_scope(f"mlp2T{q}"):
        for t in range(NT):
            p2 = ps.tile([128, D], F32, tag="ps3", bufs=2,
                         name=f"p2{q}_{t}")
            nc.tensor.matmul(p2[:, 0:512], c.ones1p[:],
                             c.b2row_t[:, 0:512], start=True, stop=False)
            nc.tensor.matmul(p2[:, 512:768], c.ones1p[:],
                             c.b2row_t[:, 512:768], start=True,
                             stop=False)
            for j in range(KH // 2):
                nc.tensor.matmul(p2[:, 0:512],
                                 s.mlpb[j][:, :, t * 128:(t + 1) * 128],
                                 c.w2[j][:, :, 0:512],
                                 start=False, stop=(j == KH // 2 - 1),
                                 perf_mode=PM.DoubleRow)
                nc.tensor.matmul(p2[:, 512:768],
                                 s.mlpb[j][:, :, t * 128:(t + 1) * 128],
                                 c.w2[j][:, :, 512:768],
                                 start=False, stop=(j == KH // 2 - 1),
                                 perf_mode=PM.DoubleRow)
            nc.vector.tensor_tensor(s.z2[t][:], p2[:], s.hbT[t][:],
                                    op=OP.add)


def _mlp2T_last(c, s, q):
    """Last chunk: mlp2 matmuls interleave with per-group LN2 + output DMA
    so the final tile's chain (matmul->add->stats->apply->DMA) is as short
    as possible.  Applies split in halves so the DMA starts earlier."""
    nc, sb, ps = c.nc, c.sb, c.ps
    s.z2 = [sb.tile([128, D], F16, tag="z2", bufs=8, name=f"z2_{q}_{t}")
            for t in range(NT)]
    yo = [sb.tile([128, D], F16, tag="yo", bufs=8, name=f"yo{q}_{t}")
          for t in range(NT)]
    for ts in ((0, 1), (2,), (3,)):
        with nc.named_scope(f"mlp2T{q}_{ts[0]}"):
            for t in ts:
                p2 = ps.tile([128, D], F32, tag="ps3", bufs=2,
                             name=f"p2{q}_{t}")
                nc.tensor.matmul(p2[:, 0:512], c.ones1p[:],
                                 c.b2row_t[:, 0:512], start=True, stop=False)
                nc.tensor.matmul(p2[:, 512:768], c.ones1p[:],
                                 c.b2row_t[:, 512:768], start=True,
                                 stop=False)
                for j in range(KH // 2):
                    nc.tensor.matmul(p2[:, 0:512],
                                     s.mlpb[j][:, :, t * 128:(t + 1) * 128],
                                     c.w2[j][:, :, 0:512],
                                     start=False, stop=(j == KH // 2 - 1),
                                     perf_mode=PM.DoubleRow)
                    nc.tensor.matmul(p2[:, 512:768],
                                     s.mlpb[j][:, :, t * 128:(t + 1) * 128],
                                     c.w2[j][:, :, 512:768],
                                     start=False, stop=(j == KH // 2 - 1),
                                     perf_mode=PM.DoubleRow)
                nc.vector.tensor_tensor(s.z2[t][:], p2[:], s.hbT[t][:],
                                        op=OP.add)
        mrs = _ln_stats(c, q, 2, s.z2, (ts,))
        with nc.named_scope(f"ln2a_{q}_{ts[0]}"):
            for (gts, MV, R) in mrs:
                for i, t in enumerate(gts):
                    for lo, hi in ((0, 512), (512, 768)):
                        nc.vector.tensor_scalar(
                            yo[t][:, lo:hi], s.z2[t][:, lo:hi],
                            MV[:, i, 0:1], R[:, i:i + 1],
                            op0=OP.subtract, op1=OP.mult)
                        nc.sync.dma_start(c.yt_d[q * NT + t][:, lo:hi],
                                          yo[t][:, lo:hi])


def _ln2_stats(c, s, q):
    s.ln2_mr = _ln_stats(c, q, 2, s.z2, ((0, 1), (2, 3)))


def _ln2_apply(c, s, q):
    nc, sb = c.nc, c.sb
    yo = [sb.tile([128, D], F16, tag="yo", bufs=8, name=f"yo{q}_{t}")
          for t in range(NT)]
    _ln_apply(c, q, 2, s.z2, s.ln2_mr, lambda t: yo[t][:],
              post=lambda t: nc.sync.dma_start(c.yt_d[q * NT + t],
                                               yo[t][:]))


# ---------------- host side ----------------

def host_prep(inputs):
    """Returns per-core input maps (weights shared)."""
    bf = ml_dtypes.bfloat16
    x = np.asarray(inputs["x"], np.float32)
    B = x.shape[0]

    f8 = ml_dtypes.float8_e4m3

    def wtile(w, kt):  # [out,in] -> transposed, tiled on contraction dim
        wt = np.ascontiguousarray(np.asarray(w, np.float32).T)  # [in, out]
        return wt.reshape(kt, 128, wt.shape[1]).astype(bf)

    def wtile8(w, kt):  # fp8 DoubleRow pairs: [kt//2, 128, 2, out]
        wt = np.ascontiguousarray(np.asarray(w, np.float32).T)
        t = wt.reshape(kt // 2, 2, 128, wt.shape[1]).transpose(0, 2, 1, 3)
        return np.ascontiguousarray(t).astype(f8)

    shared = {
        "wq": wtile8(inputs["Wq"], KD), "wk": wtile8(inputs["Wk"], KD),
        "wv": wtile8(inputs["Wv"], KD), "wp": wtile8(inputs["Wp"], KD),
        "w1": wtile8(inputs["W1"], KD), "w2": wtile8(inputs["W2"], KH),
        "bq": np.ascontiguousarray(
            np.asarray(inputs["bq"], np.float32).reshape(KD, 128).T),
        "bk": np.ascontiguousarray(
            np.asarray(inputs["bk"], np.float32).reshape(KD, 128).T),
        "b1": np.ascontiguousarray(
            np.asarray(inputs["b1"], np.float32).reshape(KH, 128).T),
        "b2row": np.asarray(inputs["b2"], np.float32).reshape(1, D).astype(bf),
        "ident": np.eye(128, dtype=bf),
    }
    # fold proj bias and (through softmax row-sums == 1) the V bias into the
    # token-major residual: bp' = bp + Wp @ bv
    bp2 = (np.asarray(inputs["bp"], np.float32)
           + np.asarray(inputs["Wp"], np.float32)
           @ np.asarray(inputs["bv"], np.float32))
    per_core = []
    for b in range(B):
        xb_t = np.ascontiguousarray(x[b].T)          # [D, S]
        m = dict(shared)
        m["xt"] = np.ascontiguousarray(
            xb_t.reshape(KD // 2, 2, 128, S).transpose(0, 2, 1, 3)).astype(f8)
        m["xres"] = np.ascontiguousarray(
            (x[b] + bp2[None, :]).reshape(NK, 128, D)).astype(bf)
        per_core.append(m)
    return per_core


def assemble_output(results):
    """results: list of per-core dicts with 'yt' [NK,128,D] fp16 -> [B,S,D] f32."""
    B = len(results)
    out = np.empty((B, S, D), np.float32)
    for b in range(B):
        out[b] = results[b]["yt"].reshape(S, D).astype(np.float32)
    return out


# ------------------------------------------------------------------ kernel --
_CACHE = {}


def kernel(**inputs):
    install()  # birpatch
    from concourse.bass_utils import run_bass_kernel_spmd

    per_core = host_prep(inputs)
    n = len(per_core)
    key = "nc%d" % n
    if key not in _CACHE:
        _nc = bass.Bass("TRN2", target_bir_lowering=False, debug=False,
                        num_devices=n)
        build(_nc, reps=1)
        _CACHE[key] = _nc
    _nc = _CACHE[key]
    res = run_bass_kernel_spmd(_nc, per_core, list(range(n)), trace=False)
    return assemble_output(res.results)

